# revision 1
# baseline (speedup 1.0000x reference)
"""GAT (2-layer, 8-head) forward on 8 Trainium2 NeuronCores via Bass/Tile.

Strategy (per sharding hint): partition nodes across 8 cores; each core owns the
edges whose destination lands in its partition, so segment-softmax/aggregation
are local. Within a core, destination nodes are bin-packed into 49 windows of
128 nodes; each window's incoming edges occupy <= 19 subtiles of 128 edge slots.
Per edge subtile: indirect-DMA row gathers fetch source features, attention
weights are computed on-chip, and a one-hot [edge x dst] matrix on the tensor
engine performs the segment-sum scatter (messages + softmax denominator in the
same PSUM accumulation). Layer 2 is transform-then-aggregate: z = elu(out1)@W2
is computed once per node, all-gathered across cores, and layer-2 attention
aggregates 33-float z rows. log_softmax is fused into the layer-2 finalize.
"""
import sys

sys.path.insert(0, "/opt/trn_rl_repo")

import numpy as np
from contextlib import ExitStack

import concourse.bass as bass
import concourse.tile as tile
from concourse import bacc, mybir
from concourse.bass_utils import run_bass_kernel_spmd

F32 = mybir.dt.float32
F32R = mybir.dt.float32r
I32 = mybir.dt.int32
AF = mybir.ActivationFunctionType
OP = mybir.AluOpType

# problem constants (hardcoded per contract)
N = 50000
E = 800000
IN_C = 128
HID = 32
HEADS = 8
OUT_C = 32
NEG = 0.2

NCORES = 8
NODES_PC = N // NCORES      # 6250
NW = 49                     # windows per core
WSLOT = 128
KSUB = 19                   # edge subtiles per window
CAP = KSUB * 128            # 2432
GPC = NW * WSLOT            # 6272
TOT = NCORES * GPC          # 50176
NCOL = NW * KSUB            # 931
NT0 = TOT // 128            # 392 phase-0 tiles
PAD_DST = 200.0


# ----------------------------------------------------------------------------
# host preprocessing
# ----------------------------------------------------------------------------

def _preprocess(edge_index):
    src = np.concatenate([edge_index[0], np.arange(N, dtype=np.int64)])
    dst = np.concatenate([edge_index[1], np.arange(N, dtype=np.int64)])
    Etot = src.shape[0]

    deg = np.bincount(dst, minlength=N)
    node_bin = np.zeros(N, dtype=np.int32)
    node_slot = np.zeros(N, dtype=np.int32)
    for c in range(NCORES):
        nodes = np.arange(c * NODES_PC, (c + 1) * NODES_PC)
        order = np.argsort(-deg[nodes], kind="stable")
        bins_edges = np.zeros(NW, dtype=np.int64)
        bins_count = np.zeros(NW, dtype=np.int64)
        for n in nodes[order]:
            d = deg[n]
            feas = (bins_count < WSLOT) & (bins_edges + d <= CAP)
            assert feas.any(), "window capacity overflow"
            b = int(np.argmin(np.where(feas, bins_edges, np.iinfo(np.int64).max)))
            node_bin[n] = b
            node_slot[n] = bins_count[b]
            bins_edges[b] += d
            bins_count[b] += 1

    node_gid = (np.arange(N) // NODES_PC) * GPC + node_bin * WSLOT + node_slot

    ecore = (dst // NODES_PC).astype(np.int64)
    ebin = node_bin[dst].astype(np.int64)
    key = ecore * NW + ebin
    eorder = np.argsort(key, kind="stable")
    key_sorted = key[eorder]
    grp_start = np.searchsorted(key_sorted, np.arange(NCORES * NW), side="left")
    pos_in_grp = np.arange(Etot) - grp_start[key_sorted]
    eslot = np.empty(Etot, dtype=np.int64)
    eslot[eorder] = pos_in_grp
    assert (eslot < CAP).all()

    src_idx = np.zeros((NCORES, 128, NCOL), dtype=np.int32)
    dstg_idx = np.zeros((NCORES, 128, NCOL), dtype=np.int32)
    dst_col = np.full((NCORES, 128, NCOL), PAD_DST, dtype=np.float32)
    p = (eslot % 128).astype(np.int64)
    colj = (ebin * KSUB + eslot // 128).astype(np.int64)
    src_idx[ecore, p, colj] = node_gid[src]
    dstg_idx[ecore, p, colj] = node_gid[dst]
    dst_col[ecore, p, colj] = node_slot[dst]

    return src_idx, dstg_idx, dst_col, node_gid


# ----------------------------------------------------------------------------
# bass program
# ----------------------------------------------------------------------------

def _build_program(timing=False, phases=(1, 1, 1)):
    nc = bacc.Bacc("TRN2", target_bir_lowering=False, debug=False,
                   num_devices=NCORES, num_swdge_queues=4)

    _gq = [0]

    def _gather(out_ap, table, idx_ap, element_offset=0):
        bi = nc.gpsimd.indirect_dma_start(
            out=out_ap, out_offset=None, in_=table,
            in_offset=bass.IndirectOffsetOnAxis(ap=idx_ap, axis=0),
            element_offset=element_offset)
        return bi

    xTw_d = nc.dram_tensor("xTw", [IN_C, TOT], F32R, kind="ExternalInput").ap()
    w1a_d = nc.dram_tensor("w1a", [IN_C, 272], F32R, kind="ExternalInput").ap()
    w2a_d = nc.dram_tensor("w2a", [128, 68], F32R, kind="ExternalInput").ap()
    iota_d = nc.dram_tensor("iota", [128, 128], F32, kind="ExternalInput").ap()
    ident_d = nc.dram_tensor("ident", [128, 128], F32, kind="ExternalInput").ap()
    b1t_d = nc.dram_tensor("b1t", [128, 256], F32, kind="ExternalInput").ap()
    b2t_d = nc.dram_tensor("b2t", [128, 32], F32, kind="ExternalInput").ap()
    sidx_d = nc.dram_tensor("sidx", [128, NCOL], I32, kind="ExternalInput").ap()
    didx_d = nc.dram_tensor("didx", [128, NCOL], I32, kind="ExternalInput").ap()
    dcol_d = nc.dram_tensor("dcol", [128, NCOL], F32, kind="ExternalInput").ap()

    out_d = nc.dram_tensor("out2", [GPC, OUT_C], F32, kind="ExternalOutput").ap()

    h1tab = nc.dram_tensor("h1tab", [TOT, 264], F32R, kind="Internal").ap()
    sdtab = nc.dram_tensor("sdtab", [TOT, 8], F32, kind="Internal").ap()
    zz_own = nc.dram_tensor("zz_own", [GPC, 34], F32R, kind="Internal").ap()
    zz_all = nc.dram_tensor("zz_all", [TOT, 34], F32R, kind="Internal",
                            addr_space="Shared").ap()

    with tile.TileContext(nc) as tc, ExitStack() as ctx:
        cons = ctx.enter_context(tc.tile_pool(name="cons", bufs=1))
        stat = ctx.enter_context(tc.tile_pool(name="stat", bufs=3))
        gath = ctx.enter_context(tc.tile_pool(name="gath", bufs=2))
        work = ctx.enter_context(tc.tile_pool(name="work", bufs=3))
        sub = ctx.enter_context(tc.tile_pool(name="sub", bufs=4))
        pp = ctx.enter_context(tc.tile_pool(name="pp", bufs=2, space="PSUM"))

        # ---- constants resident in SBUF ----
        w1a_t = cons.tile([IN_C, 272], F32R)
        nc.sync.dma_start(w1a_t[:], w1a_d)
        w2a_t = cons.tile([128, 68], F32R)
        nc.sync.dma_start(w2a_t[:], w2a_d)
        iota_t = cons.tile([128, 128], F32)
        nc.sync.dma_start(iota_t[:], iota_d)
        ident_t = cons.tile([128, 128], F32)
        nc.sync.dma_start(ident_t[:], ident_d)
        b1t_t = cons.tile([128, 256], F32)
        nc.sync.dma_start(b1t_t[:], b1t_d)
        b2t_t = cons.tile([128, 32], F32)
        nc.sync.dma_start(b2t_t[:], b2t_d)
        sidx_t = cons.tile([128, NCOL], I32)
        nc.sync.dma_start(sidx_t[:], sidx_d)
        didx_t = cons.tile([128, NCOL], I32)
        nc.sync.dma_start(didx_t[:], didx_d)
        dcol_t = cons.tile([128, NCOL], F32)
        nc.sync.dma_start(dcol_t[:], dcol_d)

        # ---- phase 0: h1 table [TOT, 264] + s_dst table [TOT, 8] ----
        for t in range(NT0 if phases[0] else 0):
            xt = stat.tile([IN_C, 128], F32R, tag="xt")
            nc.sync.dma_start(xt[:], xTw_d[:, t * 128:(t + 1) * 128])
            ph = pp.tile([128, 272], F32, tag="ph0")
            nc.tensor.matmul(ph[:], xt[:], w1a_t[:], start=True, stop=True)
            stg = work.tile([128, 272], F32R, tag="stg0")
            nc.scalar.activation(stg[:], ph[:], AF.Copy)
            nc.sync.dma_start(h1tab[t * 128:(t + 1) * 128, :], stg[:, 0:264])
            nc.sync.dma_start(sdtab[t * 128:(t + 1) * 128, :],
                              stg[:, 264:272].bitcast(F32))

        # ---- layer 1 ----
        for w in range(NW if phases[1] else 0):
            g_t = gath.tile([128, KSUB, 264], F32R, tag="g1")
            sd_t = gath.tile([128, KSUB, 8], F32, tag="sd1")
            for j in range(KSUB):
                col = w * KSUB + j
                _gather(g_t[:, j, :], h1tab, sidx_t[:, col:col + 1])
                _gather(sd_t[:, j, :], sdtab, didx_t[:, col:col + 1])
            # scores (bulk per window): e = leaky(s_src + s_dst); ex = exp(e)
            e_t = work.tile([128, KSUB, 8], F32, tag="e1")
            nc.vector.tensor_tensor(e_t[:], g_t[:, :, 256:264].bitcast(F32),
                                    sd_t[:], OP.add)
            lk_t = work.tile([128, KSUB, 8], F32, tag="lk1")
            nc.vector.scalar_tensor_tensor(lk_t[:], e_t[:], NEG, e_t[:],
                                           OP.mult, OP.max)
            msg_t = work.tile([128, KSUB, 264], F32R, tag="msg1")
            nc.scalar.activation(msg_t[:, :, 256:264], lk_t[:], AF.Exp)
            # messages (bulk): msg = h * ex  (per-head broadcast)
            nc.vector.tensor_tensor(
                msg_t[:, :, 0:256].rearrange("p k (h c) -> p k h c", h=HEADS),
                g_t[:, :, 0:256].bitcast(F32).rearrange(
                    "p k (h c) -> p k h c", h=HEADS),
                msg_t[:, :, 256:264].bitcast(F32).unsqueeze(3).broadcast_to(
                    [128, KSUB, HEADS, HID]),
                OP.mult)
            # scatter: one-hot matmuls accumulate into window PSUM
            acc = pp.tile([128, 264], F32, tag="acc")
            for j in range(KSUB):
                col = w * KSUB + j
                s_t = sub.tile([128, 128], F32R, tag="s1")
                nc.vector.tensor_scalar(s_t[:], iota_t[:],
                                        dcol_t[:, col:col + 1], None,
                                        OP.is_equal)
                nc.tensor.matmul(acc[:], s_t[:], msg_t[:, j, :],
                                 start=(j == 0), stop=(j == KSUB - 1))
            # finalize window: out1 = acc/den + b1; h2 = elu(out1)
            denc = work.tile([128, 8], F32, tag="denc")
            nc.vector.tensor_scalar(denc[:], acc[:, 256:264], 1e-30, None,
                                    OP.max)
            rden = work.tile([128, 8], F32, tag="rden")
            nc.vector.reciprocal(rden[:], denc[:])
            o1 = work.tile([128, 256], F32, tag="o1")
            nc.vector.tensor_tensor(
                o1[:].rearrange("p (h c) -> p h c", h=HEADS),
                acc[:, 0:256].rearrange("p (h c) -> p h c", h=HEADS),
                rden[:].unsqueeze(2).broadcast_to([128, HEADS, HID]),
                OP.mult)
            h2a = work.tile([128, 256], F32, tag="h2a")
            nc.vector.tensor_tensor(h2a[:], o1[:], b1t_t[:], OP.add)
            tmin = work.tile([128, 256], F32, tag="tmin")
            nc.vector.tensor_scalar(tmin[:], h2a[:], 0.0, None, OP.min)
            eexp = work.tile([128, 256], F32, tag="eexp")
            nc.scalar.activation(eexp[:], tmin[:], AF.Exp)
            rl = work.tile([128, 256], F32, tag="rl")
            nc.vector.tensor_scalar(rl[:], h2a[:], 0.0, None, OP.max)
            h2e = work.tile([128, 256], F32, tag="h2e")
            nc.vector.scalar_tensor_tensor(h2e[:], eexp[:], -1.0, rl[:],
                                           OP.add, OP.add)
            # z = h2e @ W2 (+ attention vectors) via transpose + 2 matmuls
            zps = pp.tile([128, 34], F32, tag="zps")
            for half in range(2):
                trp = pp.tile([128, 128], F32, tag="trp")
                nc.tensor.transpose(trp[:], h2e[:, half * 128:(half + 1) * 128],
                                    ident_t[:])
                h2T = sub.tile([128, 128], F32R, tag="h2T")
                nc.scalar.activation(h2T[:], trp[:], AF.Copy)
                nc.tensor.matmul(zps[:], h2T[:],
                                 w2a_t[:, half * 34:(half + 1) * 34],
                                 start=(half == 0), stop=(half == 1))
            zst = work.tile([128, 34], F32R, tag="zst")
            nc.scalar.activation(zst[:], zps[:], AF.Copy)
            nc.sync.dma_start(zz_own[w * 128:(w + 1) * 128, :], zst[:])

        # ---- all-gather z across cores ----
        if timing:
            # TimelineSim can't model collectives; stand in the same bytes
            # (each core receives NCORES slices) with plain DMAs.
            for c in range(NCORES):
                nc.sync.dma_start(zz_all[c * GPC:(c + 1) * GPC, :], zz_own)
        else:
            nc.gpsimd.collective_compute(
                "AllGather", OP.bypass,
                replica_groups=[list(range(NCORES))],
                ins=[zz_own], outs=[zz_all])

        # ---- layer 2 ----
        for w in range(NW if phases[2] else 0):
            gz_t = gath.tile([128, KSUB, 34], F32R, tag="g2")
            sd2_t = gath.tile([128, KSUB], F32, tag="sd2")
            for j in range(KSUB):
                col = w * KSUB + j
                _gather(gz_t[:, j, :], zz_all, sidx_t[:, col:col + 1])
                _gather(sd2_t[:, j:j + 1].bitcast(F32R), zz_all,
                        didx_t[:, col:col + 1], element_offset=33)
            e2_t = work.tile([128, KSUB], F32, tag="e2")
            nc.vector.tensor_tensor(e2_t[:],
                                    gz_t[:, :, 32:33].bitcast(F32).squeeze(2),
                                    sd2_t[:], OP.add)
            lk2_t = work.tile([128, KSUB], F32, tag="lk2")
            nc.vector.scalar_tensor_tensor(lk2_t[:], e2_t[:], NEG, e2_t[:],
                                           OP.mult, OP.max)
            ex2_t = work.tile([128, KSUB], F32R, tag="ex2")
            nc.scalar.activation(ex2_t[:], lk2_t[:], AF.Exp)
            nc.vector.memset(gz_t[:, :, 32:33].bitcast(F32), 1.0)

            acc2 = pp.tile([128, 34], F32, tag="acc")
            for j in range(KSUB):
                col = w * KSUB + j
                s_t = sub.tile([128, 128], F32R, tag="s2")
                nc.vector.tensor_scalar(s_t[:], iota_t[:],
                                        dcol_t[:, col:col + 1], None,
                                        OP.is_equal)
                gzs = sub.tile([128, 34], F32R, tag="gzs")
                nc.vector.tensor_scalar(gzs[:], gz_t[:, j, 0:34].bitcast(F32),
                                        ex2_t[:, j:j + 1].bitcast(F32), None,
                                        OP.mult)
                nc.tensor.matmul(acc2[:], s_t[:], gzs[:],
                                 start=(j == 0), stop=(j == KSUB - 1))
            # finalize: out2 = log_softmax(acc2/den + b2)
            den2 = work.tile([128, 1], F32, tag="den2")
            nc.vector.tensor_scalar(den2[:], acc2[:, 32:33], 1e-30, None,
                                    OP.max)
            rd2 = work.tile([128, 1], F32, tag="rd2")
            nc.vector.reciprocal(rd2[:], den2[:])
            o2 = work.tile([128, 32], F32, tag="o2")
            nc.vector.tensor_scalar(o2[:], acc2[:, 0:32], rd2[:], None, OP.mult)
            o2b = work.tile([128, 32], F32, tag="o2b")
            nc.vector.tensor_tensor(o2b[:], o2[:], b2t_t[:], OP.add)
            mx = work.tile([128, 1], F32, tag="mx")
            nc.vector.tensor_reduce(mx[:], o2b[:], mybir.AxisListType.X, OP.max)
            xm = work.tile([128, 32], F32, tag="xm")
            nc.vector.tensor_scalar(xm[:], o2b[:], mx[:], None, OP.subtract)
            ew = work.tile([128, 32], F32, tag="ew")
            ssum = work.tile([128, 1], F32, tag="ssum")
            nc.scalar.activation(ew[:], xm[:], AF.Exp, accum_out=ssum[:])
            lns = work.tile([128, 1], F32, tag="lns")
            nc.scalar.activation(lns[:], ssum[:], AF.Ln)
            fin = work.tile([128, 32], F32, tag="fin")
            nc.vector.tensor_scalar(fin[:], xm[:], lns[:], None, OP.subtract)
            nc.sync.dma_start(out_d[w * 128:(w + 1) * 128, :], fin[:])

    nc.compile()
    return nc


_CACHE = {}


def _get_program():
    if "nc" not in _CACHE:
        _CACHE["nc"] = _build_program()
    return _CACHE["nc"]


def _build_timing_program():
    return _build_program(timing=True)


def _host_arrays(inputs):
    x = np.ascontiguousarray(np.asarray(inputs["x"], dtype=np.float32))
    edge_index = np.asarray(inputs["edge_index"])
    W1 = np.asarray(inputs["W1"], dtype=np.float32)
    as1 = np.asarray(inputs["att_src1"], dtype=np.float32)
    ad1 = np.asarray(inputs["att_dst1"], dtype=np.float32)
    b1 = np.asarray(inputs["b1"], dtype=np.float32)
    W2 = np.asarray(inputs["W2"], dtype=np.float32)
    as2 = np.asarray(inputs["att_src2"], dtype=np.float32)
    ad2 = np.asarray(inputs["att_dst2"], dtype=np.float32)
    b2 = np.asarray(inputs["b2"], dtype=np.float32)

    src_idx, dstg_idx, dst_col, node_gid = _preprocess(edge_index)

    xTw = np.zeros((IN_C, TOT), np.float32)
    xTw[:, node_gid] = x.T
    A_src = (W1.reshape(IN_C, HEADS, HID) * as1[None]).sum(-1)
    A_dst = (W1.reshape(IN_C, HEADS, HID) * ad1[None]).sum(-1)
    w1a = np.concatenate([W1, A_src, A_dst], axis=1).astype(np.float32)
    a2s = W2 @ as2[0]
    a2d = W2 @ ad2[0]
    W2A2 = np.concatenate([W2, a2s[:, None], a2d[:, None]], axis=1)  # [256,34]
    w2a = np.concatenate([W2A2[0:128], W2A2[128:256]], axis=1).astype(np.float32)
    iota = np.tile(np.arange(128, dtype=np.float32), (128, 1))
    ident = np.eye(128, dtype=np.float32)
    b1t = np.tile(b1[None, :], (128, 1)).astype(np.float32)
    b2t = np.tile(b2[None, :], (128, 1)).astype(np.float32)

    in_maps = []
    for c in range(NCORES):
        in_maps.append(dict(
            xTw=xTw, w1a=w1a, w2a=w2a, iota=iota, ident=ident,
            b1t=b1t, b2t=b2t,
            sidx=src_idx[c], didx=dstg_idx[c], dcol=dst_col[c],
        ))
    return in_maps, node_gid


def kernel(**inputs):
    in_maps, node_gid = _host_arrays(inputs)
    nc = _get_program()
    res = run_bass_kernel_spmd(nc, in_maps, core_ids=list(range(NCORES)))
    out_full = np.concatenate(
        [np.asarray(res.results[c]["out2"], dtype=np.float32)
         for c in range(NCORES)], axis=0)
    return out_full[node_gid]



# revision 13
# speedup vs baseline: 2.1745x; 2.1745x over previous
"""GAT (2-layer, 8-head) forward on 8 Trainium2 NeuronCores via Bass/Tile.

Destination-major strategy: nodes are partitioned across 8 cores (contiguous
ranges of 6250). Within a core, nodes are sorted by in-degree and packed into
49 windows of 128 nodes; a window's nodes sit on SBUF partitions and their
incoming edges lie along the free axis, padded to the window max degree
(degree-sorted packing keeps padding ~3%). Windows are processed in PAIRS
with a shared padded degree Kg so every op covers [128, 2, Kg, ...]:
  - ONE batched indirect DMA per pair gathers all [128, 2*Kg] source rows
    (h | s_src | s_dst packed per row, f16, 544B) -- no per-edge dst-score
    gathers and no one-hot scatter matmuls at all.
  - dst attention scores are per-partition values (broadcast for free).
  - messages are formed in place over the gathered rows and segment-summed
    by an in-place binary-fold over the edge axis (packed f16 adds).
Layer-1 node features h1 = x@W1 (+ fused score vectors) are computed
replicated on every core into a local f16 table; layer 2 aggregates 34-float
z rows that are AllGather'ed across cores. log_softmax is fused into the
layer-2 finalize.
"""
import sys

sys.path.insert(0, "/opt/trn_rl_repo")

import numpy as np
from contextlib import ExitStack

import concourse.bass as bass
import concourse.tile as tile
from concourse import bacc, mybir
from concourse.bass_utils import run_bass_kernel_spmd

F16 = mybir.dt.float16
F32 = mybir.dt.float32
F32R = mybir.dt.float32r
I32 = mybir.dt.int32
AF = mybir.ActivationFunctionType
OP = mybir.AluOpType

# problem constants (hardcoded per contract)
N = 50000
E = 800000
IN_C = 128
HID = 32
HEADS = 8
OUT_C = 32
NEG = 0.2

NCORES = 8
NPC = N // NCORES           # 6250 nodes per core
NW = 49                     # windows per core
GPC = NW * 128              # 6272 slots per core (incl 22 pads in window 48)
TOT = NCORES * GPC          # 50176
NT0 = TOT // 128            # 392 phase-0 tiles
NGRP = (NW + 1) // 2        # 25 window pairs (last one is a singleton)
PAD_FILL = -200.0           # pad source score -> exp(leaky(.)) flushes to 0
PADP = NPC - 48 * 128       # 106: first pad partition of window 48
ROW1 = HEADS * HID + 2 * HEADS   # 272: h(256) | s_src(8) | s_dst(8)
ROW2 = OUT_C + 2                 # 34: z(32) | s_src2 | s_dst2


# ----------------------------------------------------------------------------
# host preprocessing
# ----------------------------------------------------------------------------

def _preprocess(edge_index):
    src = np.concatenate([np.asarray(edge_index[0], np.int64),
                          np.arange(N, dtype=np.int64)])
    dst = np.concatenate([np.asarray(edge_index[1], np.int64),
                          np.arange(N, dtype=np.int64)])
    deg = np.bincount(dst, minlength=N)          # >= 1 (self-loops)

    gid = np.empty(N, np.int64)
    K_cw = np.zeros((NCORES, NW), np.int64)
    for c in range(NCORES):
        d = deg[c * NPC:(c + 1) * NPC]
        order = np.argsort(-d, kind="stable")
        ranks = np.empty(NPC, np.int64)
        ranks[order] = np.arange(NPC)
        gid[c * NPC:(c + 1) * NPC] = c * GPC + ranks
        ds = d[order]                            # descending
        K_cw[c] = ds[np.minimum(np.arange(NW) * 128, NPC - 1)]
    K_w = np.maximum(K_cw.max(axis=0), 1)
    # pair windows (2g, 2g+1) under a shared padded degree Kg
    K_g = np.array([max(K_w[2 * g], K_w[2 * g + 1]) if 2 * g + 1 < NW
                    else K_w[2 * g] for g in range(NGRP)], np.int64)
    goff = np.zeros(NGRP + 1, np.int64)
    goff[1:] = np.cumsum(2 * K_g)
    NCOL = int(goff[-1])

    # place every edge at (core, dst_slot, goff[g] + j*Kg + rank_in_node)
    Etot = dst.shape[0]
    eorder = np.argsort(dst, kind="stable")
    dst_s = dst[eorder]
    src_s = src[eorder]
    row_start = np.zeros(N, np.int64)
    row_start[1:] = np.cumsum(deg)[:-1]
    pos = np.arange(Etot) - row_start[dst_s]
    c_e = dst_s // NPC
    loc = gid[dst_s] - c_e * GPC
    w_e = loc // 128
    s_e = loc % 128
    g_e = w_e // 2
    col = goff[g_e] + (w_e % 2) * K_g[g_e] + pos
    assert (pos < K_w[w_e]).all()

    sidx = np.empty((NCORES, 128, NCOL), np.int32)
    for c in range(NCORES):
        sidx[c] = c * GPC + GPC - 1              # dummy pad row (h=0, s=-200)
    sidx[c_e, s_e, col] = gid[src_s].astype(np.int32)

    ownidx = np.empty((NCORES, 128, NW + 1), np.int32)
    base = (np.arange(NW)[None, :] * 128 + np.arange(128)[:, None])
    for c in range(NCORES):
        ownidx[c, :, 0:NW] = c * GPC + base
        ownidx[c, :, NW] = c * GPC + GPC - 1     # phantom sub-window
    return sidx, ownidx, gid, tuple(int(k) for k in K_g)


# ----------------------------------------------------------------------------
# bass program
# ----------------------------------------------------------------------------

def _build_program(K_g, timing=False, phases=(1, 1, 1)):
    KGMAX = max(K_g)
    goff = np.concatenate([[0], np.cumsum([2 * k for k in K_g])]).astype(int)
    NCOL = int(goff[-1])

    nc = bacc.Bacc("TRN2", target_bir_lowering=False, debug=False,
                   num_devices=NCORES, num_swdge_queues=4)

    def _gather(out_ap, table, idx_ap, element_offset=0):
        return nc.gpsimd.indirect_dma_start(
            out=out_ap, out_offset=None, in_=table,
            in_offset=bass.IndirectOffsetOnAxis(ap=idx_ap, axis=0),
            element_offset=element_offset)

    def stt(out, in0, scalar, in1, op0, op1):
        nc.vector.scalar_tensor_tensor(out, in0, scalar, in1, op0, op1)

    xTw_d = nc.dram_tensor("xTw", [IN_C, TOT], F16, kind="ExternalInput").ap()
    w1a_d = nc.dram_tensor("w1a", [IN_C, ROW1], F16, kind="ExternalInput").ap()
    w2a_d = nc.dram_tensor("w2a", [128, 2 * ROW2], F32R, kind="ExternalInput").ap()
    ident_d = nc.dram_tensor("ident", [128, 128], F32, kind="ExternalInput").ap()
    b1t_d = nc.dram_tensor("b1t", [128, 256], F16, kind="ExternalInput").ap()
    b2t_d = nc.dram_tensor("b2t", [128, 32], F32, kind="ExternalInput").ap()
    sidx_d = nc.dram_tensor("sidx", [128, NCOL], I32, kind="ExternalInput").ap()
    oidx_d = nc.dram_tensor("oidx", [128, NW + 1], I32, kind="ExternalInput").ap()

    out_d = nc.dram_tensor("out2", [GPC, OUT_C], F32, kind="ExternalOutput").ap()

    h1tab = nc.dram_tensor("h1tab", [TOT, ROW1], F16, kind="Internal").ap()
    zz_own = nc.dram_tensor("zz_own", [GPC, ROW2], F16, kind="Internal").ap()
    zz_all = nc.dram_tensor("zz_all", [TOT, ROW2], F16, kind="Internal",
                            addr_space="Shared").ap()

    with tile.TileContext(nc) as tc, ExitStack() as ctx:
        cons = ctx.enter_context(tc.tile_pool(name="cons", bufs=1))
        stat = ctx.enter_context(tc.tile_pool(name="stat", bufs=3))
        gath = ctx.enter_context(tc.tile_pool(name="gath", bufs=2))
        work = ctx.enter_context(tc.tile_pool(name="work", bufs=3))
        sub = ctx.enter_context(tc.tile_pool(name="sub", bufs=3))
        pp = ctx.enter_context(tc.tile_pool(name="pp", bufs=2, space="PSUM"))

        # ---- constants resident in SBUF ----
        w1a_t = cons.tile([IN_C, ROW1], F16)
        nc.sync.dma_start(w1a_t[:], w1a_d)
        w2a_t = cons.tile([128, 2 * ROW2], F32R)
        nc.sync.dma_start(w2a_t[:], w2a_d)
        ident_t = cons.tile([128, 128], F32)
        nc.sync.dma_start(ident_t[:], ident_d)
        b1t_t = cons.tile([128, 256], F16)
        nc.sync.dma_start(b1t_t[:], b1t_d)
        b2t_t = cons.tile([128, 32], F32)
        nc.sync.dma_start(b2t_t[:], b2t_d)
        sidx_t = cons.tile([128, NCOL], I32)
        nc.sync.dma_start(sidx_t[:], sidx_d)
        oidx_t = cons.tile([128, NW + 1], I32)
        nc.sync.dma_start(oidx_t[:], oidx_d)
        sdown = cons.tile([128, NW + 1, HEADS], F16)   # own s_dst, layer 1
        sd2own = cons.tile([128, NW + 1], F32)         # own s_dst, layer 2
        num2a = cons.tile([128, NW + 1, 32], F32)      # layer-2 numerators
        den2a = cons.tile([128, NW + 1], F32)          # layer-2 denominators
        padc = cons.tile([128, 16], F16)
        nc.vector.memset(padc[:], PAD_FILL)

        # ---- phase 0: h1 table [TOT, 272] f16 = [h | s_src | s_dst] ----
        # 8 node-tiles per DMA load, PSUM/copy/store in pairs.
        for t8 in range(NT0 // 8 if phases[0] else 0):
            xt8 = stat.tile([IN_C, 8, 128], F16, tag="xt8")
            nc.sync.dma_start(xt8[:], xTw_d[:, t8 * 1024:(t8 + 1) * 1024])
            for q in range(4):
                t = t8 * 8 + 2 * q
                ph2 = pp.tile([128, 2, 512], F32, tag="ph0")
                for i in range(2):
                    nc.tensor.matmul(ph2[:, i, 0:ROW1], xt8[:, 2 * q + i, :],
                                     w1a_t[:], start=True, stop=True)
                stg2 = work.tile([128, 2, ROW1], F16, tag="stg0")
                nc.scalar.activation(stg2[:], ph2[:, :, 0:ROW1], AF.Copy)
                nc.sync.dma_start(
                    h1tab[t * 128:(t + 2) * 128, :]
                    .rearrange("(i p) c -> p i c", i=2), stg2[:])

        if phases[0]:
            for c in range(NCORES):
                nc.sync.dma_start(
                    h1tab[c * GPC + NPC:(c + 1) * GPC, 256:ROW1],
                    padc[0:GPC - NPC, :])

        if phases[1]:
            # own per-node dst scores -> SBUF (one tiny batched gather)
            for w in range(NW + 1):
                _gather(sdown[:, w, :], h1tab, oidx_t[:, w:w + 1],
                        element_offset=264)

        # ---- layer 1 (window pairs) ----
        for gi in range(NGRP if phases[1] else 0):
            Kg = K_g[gi]
            off = int(goff[gi])
            w0 = 2 * gi
            GS = 1 if gi == NGRP - 1 else 2
            K2 = 2 * Kg
            g = gath.tile([128, 2 * KGMAX, ROW1], F16, tag="g1")
            for col in range(K2):
                _gather(g[:, col, :], h1tab, sidx_t[:, off + col:off + col + 1])
            # e = s_src[src] + s_dst[dst] ; leaky in place; exp on Act
            e = work.tile([128, 2 * KGMAX, HEADS], F16, tag="e1")
            nc.vector.tensor_tensor(
                e[:, 0:K2, :].rearrange("p (g k) h -> p g k h", g=2),
                g[:, 0:K2, 256:264].rearrange("p (g k) h -> p g k h", g=2),
                sdown[:, w0:w0 + 2, :].unsqueeze(2).broadcast_to(
                    [128, 2, Kg, HEADS]),
                OP.add)
            stt(e[:, 0:K2, :], e[:, 0:K2, :], NEG, e[:, 0:K2, :],
                OP.mult, OP.max)
            ex = work.tile([128, 2 * KGMAX, HEADS], F16, tag="ex1")
            nc.scalar.activation(ex[:, 0:K2, :], e[:, 0:K2, :], AF.Exp)
            # msg = h * ex in place over the gathered rows; h is stored
            # channel-major/head-minor so the broadcast stays off the packed
            # innermost axis (keeps the 2x DVE mode)
            nc.vector.tensor_tensor(
                g[:, 0:K2, 0:256].rearrange("p k (c h) -> p k c h", h=HEADS),
                g[:, 0:K2, 0:256].rearrange("p k (c h) -> p k c h", h=HEADS),
                ex[:, 0:K2, :].unsqueeze(2).broadcast_to(
                    [128, K2, HID, HEADS]),
                OP.mult)
            # segment sum: in-place binary fold over the edge axis
            gv = g[:, 0:K2, 0:256].rearrange("p (g k) c -> p g k c", g=2)
            s = Kg
            while s > 1:
                h = s - s // 2
                nc.vector.tensor_tensor(gv[:, :, 0:s // 2, :],
                                        gv[:, :, 0:s // 2, :],
                                        gv[:, :, h:s, :], OP.add)
                s = h
            den = work.tile([128, 2, HEADS], F32, tag="den")
            nc.vector.tensor_reduce(
                den[:], ex[:, 0:K2, :].rearrange("p (g k) h -> p g h k", g=2),
                mybir.AxisListType.X, OP.add)
            denc = work.tile([128, 2, HEADS], F32, tag="denc")
            nc.vector.tensor_scalar(denc[:], den[:], 1e-4, None, OP.max)
            rden = work.tile([128, 2, HEADS], F16, tag="rden")
            with nc.allow_low_precision(reason="1/den rounded to f16; dens "
                                        "are clamped >= 1e-4 so f16 is safe"):
                nc.vector.reciprocal(rden[:], denc[:])
            o1 = work.tile([128, 2, 256], F16, tag="o1")
            nc.vector.tensor_tensor(
                o1[:].rearrange("p g (c h) -> p g c h", h=HEADS),
                gv[:, :, 0, :].rearrange("p g (c h) -> p g c h", h=HEADS),
                rden[:].unsqueeze(2).broadcast_to([128, 2, HID, HEADS]),
                OP.mult)
            h2a = work.tile([128, 2, 256], F16, tag="h2a")
            nc.vector.tensor_tensor(
                h2a[:], o1[:],
                b1t_t[:].unsqueeze(1).broadcast_to([128, 2, 256]), OP.add)
            # elu(x) = exp(min(x,0)) - 1 + x - min(x,0)
            tmin = work.tile([128, 2, 256], F16, tag="tmin")
            nc.vector.tensor_scalar(tmin[:], h2a[:], 0.0, None, OP.min)
            eexp = work.tile([128, 2, 256], F16, tag="eexp")
            nc.scalar.activation(eexp[:], tmin[:], AF.Exp)
            t1 = work.tile([128, 2, 256], F16, tag="t1")
            nc.vector.tensor_tensor(t1[:], eexp[:], h2a[:], OP.add)
            h2e = work.tile([128, 2, 256], F32, tag="h2e")
            stt(h2e[:], t1[:], -1.0, tmin[:], OP.add, OP.subtract)
            # z = h2e @ W2 (+ attention vectors) via transpose + 2 matmuls
            zst2 = work.tile([128, 2, ROW2], F16, tag="zst")
            for i in range(2):
                zps = pp.tile([128, ROW2], F32, tag="zps")
                for half in range(2):
                    trp = pp.tile([128, 128], F32, tag="trp")
                    nc.tensor.transpose(
                        trp[:], h2e[:, i, half * 128:(half + 1) * 128],
                        ident_t[:])
                    h2T = sub.tile([128, 128], F32R, tag="h2T")
                    nc.scalar.activation(h2T[:], trp[:], AF.Copy)
                    nc.tensor.matmul(zps[:], h2T[:],
                                     w2a_t[:, half * ROW2:(half + 1) * ROW2],
                                     start=(half == 0), stop=(half == 1))
                nc.scalar.activation(zst2[:, i, :], zps[:], AF.Copy)
            nc.vector.tensor_copy(sd2own[:, w0:w0 + 2], zst2[:, :, 33])
            nc.sync.dma_start(
                zz_own[w0 * 128:(w0 + GS) * 128, :]
                .rearrange("(i p) c -> p i c", i=GS), zst2[:, 0:GS, :])

        if phases[1]:
            nc.sync.dma_start(zz_own[NPC:GPC, 32:33], padc[0:GPC - NPC, 0:1])

        # ---- all-gather z across cores ----
        if phases[2]:
            if timing:
                # TimelineSim can't model collectives; stand in the same
                # bytes (each core receives NCORES slices) with plain DMAs.
                for c in range(NCORES):
                    nc.sync.dma_start(zz_all[c * GPC:(c + 1) * GPC, :], zz_own)
            else:
                nc.gpsimd.collective_compute(
                    "AllGather", OP.bypass,
                    replica_groups=[list(range(NCORES))],
                    ins=[zz_own], outs=[zz_all])

        # ---- layer 2 (window pairs) ----
        for gi in range(NGRP if phases[2] else 0):
            Kg = K_g[gi]
            off = int(goff[gi])
            w0 = 2 * gi
            GS = 1 if gi == NGRP - 1 else 2
            K2 = 2 * Kg
            zg = gath.tile([128, 2 * KGMAX, ROW2], F16, tag="g2")
            for col in range(K2):
                _gather(zg[:, col, :], zz_all, sidx_t[:, off + col:off + col + 1])
            e2 = work.tile([128, 2 * KGMAX], F32, tag="e2")
            stt(e2[:, 0:K2].rearrange("p (g k) -> p g k", g=2),
                zg[:, 0:K2, 32].rearrange("p (g k) -> p g k", g=2),
                0.0,
                sd2own[:, w0:w0 + 2].unsqueeze(2).broadcast_to([128, 2, Kg]),
                OP.add, OP.add)
            stt(e2[:, 0:K2], e2[:, 0:K2], NEG, e2[:, 0:K2], OP.mult, OP.max)
            ex2 = work.tile([128, 2 * KGMAX], F16, tag="ex2")
            nc.scalar.activation(ex2[:, 0:K2], e2[:, 0:K2], AF.Exp)
            stt(zg[:, 0:K2, 0:32], zg[:, 0:K2, 0:32], 0.0,
                ex2[:, 0:K2].unsqueeze(2).broadcast_to([128, K2, 32]),
                OP.add, OP.mult)
            nc.vector.tensor_reduce(
                num2a[:, w0:w0 + 2, :],
                zg[:, 0:K2, 0:32].rearrange("p (g k) c -> p g c k", g=2),
                mybir.AxisListType.X, OP.add)
            nc.vector.tensor_reduce(
                den2a[:, w0:w0 + 2],
                ex2[:, 0:K2].rearrange("p (g k) -> p g k", g=2),
                mybir.AxisListType.X, OP.add)

        if phases[2]:
            # batched finalize: normalize + bias + log_softmax for all
            # windows at once (keeps Ln off the per-pair Act hot path)
            NWP = NW + 1
            nc.vector.tensor_scalar(den2a[:], den2a[:], 1e-30, None, OP.max)
            rd2 = work.tile([128, NWP], F32, tag="rd2")
            nc.vector.reciprocal(rd2[:], den2a[:])
            stt(num2a[:], num2a[:], 0.0,
                rd2[:].unsqueeze(2).broadcast_to([128, NWP, 32]),
                OP.add, OP.mult)
            stt(num2a[:], num2a[:], 0.0,
                b2t_t[:].unsqueeze(1).broadcast_to([128, NWP, 32]),
                OP.add, OP.add)
            mx = work.tile([128, NWP], F32, tag="mx")
            nc.vector.tensor_reduce(mx[:], num2a[:], mybir.AxisListType.X,
                                    OP.max)
            stt(num2a[:], num2a[:], 0.0,
                mx[:].unsqueeze(2).broadcast_to([128, NWP, 32]),
                OP.add, OP.subtract)
            ew = work.tile([128, NWP, 32], F32, tag="ew")
            nc.scalar.activation(ew[:], num2a[:], AF.Exp)
            ssum = work.tile([128, NWP], F32, tag="ssum")
            nc.vector.tensor_reduce(ssum[:], ew[:], mybir.AxisListType.X,
                                    OP.add)
            lns = work.tile([128, NWP], F32, tag="lns")
            nc.scalar.activation(lns[:], ssum[:], AF.Ln)
            stt(num2a[:], num2a[:], 0.0,
                lns[:].unsqueeze(2).broadcast_to([128, NWP, 32]),
                OP.add, OP.subtract)
            nc.sync.dma_start(
                out_d[:].rearrange("(w p) c -> p w c", p=128),
                num2a[:, 0:NW, :])

    nc.compile()
    return nc


_CACHE = {}


def _get_program(K_g):
    key = ("nc", K_g)
    if key not in _CACHE:
        _CACHE[key] = _build_program(K_g)
    return _CACHE[key]


def _build_timing_program():
    K_g = _CACHE.get("K_g")
    assert K_g is not None, "call kernel() before _build_timing_program()"
    return _build_program(K_g, timing=True)


def _host_arrays(inputs):
    x = np.ascontiguousarray(np.asarray(inputs["x"], dtype=np.float32))
    edge_index = np.asarray(inputs["edge_index"])
    W1 = np.asarray(inputs["W1"], dtype=np.float32)
    as1 = np.asarray(inputs["att_src1"], dtype=np.float32)
    ad1 = np.asarray(inputs["att_dst1"], dtype=np.float32)
    b1 = np.asarray(inputs["b1"], dtype=np.float32)
    W2 = np.asarray(inputs["W2"], dtype=np.float32)
    as2 = np.asarray(inputs["att_src2"], dtype=np.float32)
    ad2 = np.asarray(inputs["att_dst2"], dtype=np.float32)
    b2 = np.asarray(inputs["b2"], dtype=np.float32)

    sidx, ownidx, gid, K_g = _preprocess(edge_index)

    xTw = np.zeros((IN_C, TOT), np.float16)
    xTw[:, gid] = x.T.astype(np.float16)
    # hidden features are stored channel-major/head-minor on device
    W1cm = (W1.reshape(IN_C, HEADS, HID).transpose(0, 2, 1)
            .reshape(IN_C, HEADS * HID))
    A_src = (W1.reshape(IN_C, HEADS, HID) * as1[None]).sum(-1)
    A_dst = (W1.reshape(IN_C, HEADS, HID) * ad1[None]).sum(-1)
    w1a = np.concatenate([W1cm, A_src, A_dst], axis=1).astype(np.float16)
    a2s = W2 @ as2[0]
    a2d = W2 @ ad2[0]
    W2A2 = np.concatenate([W2, a2s[:, None], a2d[:, None]], axis=1)  # [256,34]
    W2A2 = (W2A2.reshape(HEADS, HID, ROW2).transpose(1, 0, 2)
            .reshape(HEADS * HID, ROW2))                # c-major rows
    w2a = np.concatenate([W2A2[0:128], W2A2[128:256]], axis=1).astype(np.float32)
    ident = np.eye(128, dtype=np.float32)
    b1cm = b1.reshape(HEADS, HID).T.reshape(-1)
    b1t = np.tile(b1cm[None, :], (128, 1)).astype(np.float16)
    b2t = np.tile(b2[None, :], (128, 1)).astype(np.float32)

    in_maps = []
    for c in range(NCORES):
        in_maps.append(dict(
            xTw=xTw, w1a=w1a, w2a=w2a, ident=ident, b1t=b1t, b2t=b2t,
            sidx=sidx[c], oidx=ownidx[c],
        ))
    return in_maps, gid, K_g


def kernel(**inputs):
    in_maps, gid, K_g = _host_arrays(inputs)
    _CACHE["K_g"] = K_g
    nc = _get_program(K_g)
    res = run_bass_kernel_spmd(nc, in_maps, core_ids=list(range(NCORES)))
    out_full = np.concatenate(
        [np.asarray(res.results[c]["out2"], dtype=np.float32)
         for c in range(NCORES)], axis=0)
    return out_full[gid]


# revision 15
# speedup vs baseline: 2.2952x; 1.0555x over previous
"""GAT (2-layer, 8-head) forward on 8 Trainium2 NeuronCores via Bass/Tile.

Destination-major strategy: nodes are partitioned across 8 cores (contiguous
ranges of 6250). Within a core, nodes are sorted by in-degree and packed into
49 windows of 128 nodes; a window's nodes sit on SBUF partitions and their
incoming edges lie along the free axis, padded to the window max degree
(degree-sorted packing keeps padding ~3%). Windows are processed in PAIRS
with a shared padded degree Kg so every compute op covers [128, 2, Kg, ...]:
  - indirect row gathers fetch [h | s_src | s_dst] source rows (f16, 544B)
    one 128-index column at a time (the HW SWDGE supports exactly one index
    per partition per indirect DMA; its ~1us/instr prep is the bottleneck).
  - dst attention scores are per-partition values (broadcast for free) --
    no per-edge dst-score gathers and no one-hot scatter matmuls at all.
  - messages are formed in place over the gathered rows (h is stored
    channel-major so the ex broadcast stays off the packed innermost axis,
    keeping the 2x DVE mode) and segment-summed by an in-place binary fold
    over the edge axis.
Layer-1 node features h1 = x@W1 (+ fused score vectors) are computed
replicated on every core into a local f16 table; layer 2 aggregates 34-float
z rows that are AllGather'ed across cores. log_softmax runs as one batched
epilogue over all windows (keeps Ln off the per-pair Act hot path).
"""
import sys

sys.path.insert(0, "/opt/trn_rl_repo")

import numpy as np
from contextlib import ExitStack

import concourse.bass as bass
import concourse.tile as tile
from concourse import bacc, mybir
from concourse.bass_utils import run_bass_kernel_spmd

F16 = mybir.dt.float16
F32 = mybir.dt.float32
F32R = mybir.dt.float32r
I32 = mybir.dt.int32
AF = mybir.ActivationFunctionType
OP = mybir.AluOpType

# problem constants (hardcoded per contract)
N = 50000
E = 800000
IN_C = 128
HID = 32
HEADS = 8
OUT_C = 32
NEG = 0.2

NCORES = 8
NPC = N // NCORES           # 6250 nodes per core
NW = 49                     # windows per core
GPC = NW * 128              # 6272 slots per core (incl 22 pads in window 48)
TOT = NCORES * GPC          # 50176
NT0 = TOT // 128            # 392 phase-0 tiles
NGRP = (NW + 1) // 2        # 25 window pairs (last one is a singleton)
PAD_FILL = -200.0           # pad source score -> exp(leaky(.)) flushes to 0
PADP = NPC - 48 * 128       # 106: first pad partition of window 48
ROW1 = HEADS * HID + 2 * HEADS   # 272: h(256) | s_src(8) | s_dst(8)
ROW2 = OUT_C + 2                 # 34: z(32) | s_src2 | s_dst2


# ----------------------------------------------------------------------------
# host preprocessing
# ----------------------------------------------------------------------------

def _preprocess(edge_index):
    src = np.concatenate([np.asarray(edge_index[0], np.int64),
                          np.arange(N, dtype=np.int64)])
    dst = np.concatenate([np.asarray(edge_index[1], np.int64),
                          np.arange(N, dtype=np.int64)])
    deg = np.bincount(dst, minlength=N)          # >= 1 (self-loops)

    gid = np.empty(N, np.int64)
    K_cw = np.zeros((NCORES, NW), np.int64)
    for c in range(NCORES):
        d = deg[c * NPC:(c + 1) * NPC]
        order = np.argsort(-d, kind="stable")
        ranks = np.empty(NPC, np.int64)
        ranks[order] = np.arange(NPC)
        gid[c * NPC:(c + 1) * NPC] = c * GPC + ranks
        ds = d[order]                            # descending
        K_cw[c] = ds[np.minimum(np.arange(NW) * 128, NPC - 1)]
    K_w = np.maximum(K_cw.max(axis=0), 1)
    # pair windows (2g, 2g+1) under a shared padded degree Kg
    K_g = np.array([max(K_w[2 * g], K_w[2 * g + 1]) if 2 * g + 1 < NW
                    else K_w[2 * g] for g in range(NGRP)], np.int64)
    goff = np.zeros(NGRP + 1, np.int64)
    goff[1:] = np.cumsum(2 * K_g)
    NCOL = int(goff[-1])

    # place every edge at (core, dst_slot, goff[g] + j*Kg + rank_in_node)
    Etot = dst.shape[0]
    eorder = np.argsort(dst, kind="stable")
    dst_s = dst[eorder]
    src_s = src[eorder]
    row_start = np.zeros(N, np.int64)
    row_start[1:] = np.cumsum(deg)[:-1]
    pos = np.arange(Etot) - row_start[dst_s]
    c_e = dst_s // NPC
    loc = gid[dst_s] - c_e * GPC
    w_e = loc // 128
    s_e = loc % 128
    g_e = w_e // 2
    col = goff[g_e] + (w_e % 2) * K_g[g_e] + pos
    assert (pos < K_w[w_e]).all()

    sidx = np.empty((NCORES, 128, NCOL), np.int32)
    for c in range(NCORES):
        sidx[c] = c * GPC + GPC - 1              # dummy pad row (h=0, s=-200)
    sidx[c_e, s_e, col] = gid[src_s].astype(np.int32)

    ownidx = np.empty((NCORES, 128, NW + 1), np.int32)
    base = (np.arange(NW)[None, :] * 128 + np.arange(128)[:, None])
    for c in range(NCORES):
        ownidx[c, :, 0:NW] = c * GPC + base
        ownidx[c, :, NW] = c * GPC + GPC - 1     # phantom sub-window
    return sidx, ownidx, gid, tuple(int(k) for k in K_g)


# ----------------------------------------------------------------------------
# bass program
# ----------------------------------------------------------------------------

def _build_program(K_g, timing=False, phases=(1, 1, 1)):
    KGMAX = max(K_g)
    goff = np.concatenate([[0], np.cumsum([2 * k for k in K_g])]).astype(int)
    NCOL = int(goff[-1])

    nc = bacc.Bacc("TRN2", target_bir_lowering=False, debug=False,
                   num_devices=NCORES, num_swdge_queues=4)

    def _gather(out_ap, table, idx_ap, element_offset=0):
        return nc.gpsimd.indirect_dma_start(
            out=out_ap, out_offset=None, in_=table,
            in_offset=bass.IndirectOffsetOnAxis(ap=idx_ap, axis=0),
            element_offset=element_offset)

    def stt(out, in0, scalar, in1, op0, op1):
        nc.vector.scalar_tensor_tensor(out, in0, scalar, in1, op0, op1)

    xTw_d = nc.dram_tensor("xTw", [IN_C, TOT], F16, kind="ExternalInput").ap()
    w1a_d = nc.dram_tensor("w1a", [IN_C, ROW1], F16, kind="ExternalInput").ap()
    w2a_d = nc.dram_tensor("w2a", [128, 2 * ROW2], F32R, kind="ExternalInput").ap()
    ident_d = nc.dram_tensor("ident", [128, 128], F32, kind="ExternalInput").ap()
    b1t_d = nc.dram_tensor("b1t", [128, 256], F16, kind="ExternalInput").ap()
    b2t_d = nc.dram_tensor("b2t", [128, 32], F32, kind="ExternalInput").ap()
    sidx_d = nc.dram_tensor("sidx", [128, NCOL], I32, kind="ExternalInput").ap()
    oidx_d = nc.dram_tensor("oidx", [128, NW + 1], I32, kind="ExternalInput").ap()

    out_d = nc.dram_tensor("out2", [GPC, OUT_C], F32, kind="ExternalOutput").ap()

    h1tab = nc.dram_tensor("h1tab", [TOT, ROW1], F16, kind="Internal").ap()
    zz_own = nc.dram_tensor("zz_own", [GPC, ROW2], F16, kind="Internal").ap()
    zz_all = nc.dram_tensor("zz_all", [TOT, ROW2], F16, kind="Internal",
                            addr_space="Shared").ap()

    with tile.TileContext(nc) as tc, ExitStack() as ctx:
        cons = ctx.enter_context(tc.tile_pool(name="cons", bufs=1))
        stat = ctx.enter_context(tc.tile_pool(name="stat", bufs=3))
        gath = ctx.enter_context(tc.tile_pool(name="gath", bufs=2))
        work = ctx.enter_context(tc.tile_pool(name="work", bufs=3))
        sub = ctx.enter_context(tc.tile_pool(name="sub", bufs=3))
        pp = ctx.enter_context(tc.tile_pool(name="pp", bufs=2, space="PSUM"))

        # ---- constants resident in SBUF ----
        w1a_t = cons.tile([IN_C, ROW1], F16)
        nc.sync.dma_start(w1a_t[:], w1a_d)
        w2a_t = cons.tile([128, 2 * ROW2], F32R)
        nc.sync.dma_start(w2a_t[:], w2a_d)
        ident_t = cons.tile([128, 128], F32)
        nc.sync.dma_start(ident_t[:], ident_d)
        b1t_t = cons.tile([128, 256], F16)
        nc.sync.dma_start(b1t_t[:], b1t_d)
        b2t_t = cons.tile([128, 32], F32)
        nc.sync.dma_start(b2t_t[:], b2t_d)
        sidx_t = cons.tile([128, NCOL], I32)
        nc.sync.dma_start(sidx_t[:], sidx_d)
        oidx_t = cons.tile([128, NW + 1], I32)
        nc.sync.dma_start(oidx_t[:], oidx_d)
        sdown = cons.tile([128, NW + 1, HEADS], F16)   # own s_dst, layer 1
        sd2own = cons.tile([128, NW + 1], F32)         # own s_dst, layer 2
        num2a = cons.tile([128, NW + 1, 32], F32)      # layer-2 numerators
        den2a = cons.tile([128, NW + 1], F32)          # layer-2 denominators
        padc = cons.tile([128, 16], F16)
        nc.vector.memset(padc[:], PAD_FILL)

        # ---- phase 0: h1 table [TOT, 272] f16 = [h | s_src | s_dst] ----
        # 8 node-tiles per DMA load and per DMA store; PSUM in pairs; the
        # PSUM->SBUF f16 copies alternate between Act and DVE.
        for t8 in range(NT0 // 8 if phases[0] else 0):
            xt8 = stat.tile([IN_C, 8, 128], F16, tag="xt8")
            nc.sync.dma_start(xt8[:], xTw_d[:, t8 * 1024:(t8 + 1) * 1024])
            stg8 = work.tile([128, 8, ROW1], F16, tag="stg0")
            for q in range(4):
                ph2 = pp.tile([128, 2, 512], F32, tag="ph0")
                for i in range(2):
                    nc.tensor.matmul(ph2[:, i, 0:ROW1], xt8[:, 2 * q + i, :],
                                     w1a_t[:], start=True, stop=True)
                if q % 2 == 0:
                    nc.scalar.activation(stg8[:, 2 * q:2 * q + 2, :],
                                         ph2[:, :, 0:ROW1], AF.Copy)
                else:
                    nc.vector.tensor_copy(stg8[:, 2 * q:2 * q + 2, :],
                                          ph2[:, :, 0:ROW1])
            nc.sync.dma_start(
                h1tab[t8 * 1024:(t8 + 1) * 1024, :]
                .rearrange("(i p) c -> p i c", i=8), stg8[:])

        if phases[0]:
            for c in range(NCORES):
                nc.sync.dma_start(
                    h1tab[c * GPC + NPC:(c + 1) * GPC, 256:ROW1],
                    padc[0:GPC - NPC, :])

        if phases[1]:
            # own per-node dst scores -> SBUF (one tiny batched gather)
            for w in range(NW + 1):
                _gather(sdown[:, w, :], h1tab, oidx_t[:, w:w + 1],
                        element_offset=264)

        # ---- layer 1 (window pairs) ----
        for gi in range(NGRP if phases[1] else 0):
            Kg = K_g[gi]
            off = int(goff[gi])
            w0 = 2 * gi
            GS = 1 if gi == NGRP - 1 else 2
            K2 = 2 * Kg
            g = gath.tile([128, 2 * KGMAX, ROW1], F16, tag="g1")
            for col in range(K2):
                _gather(g[:, col, :], h1tab, sidx_t[:, off + col:off + col + 1])
            # e = s_src[src] + s_dst[dst] ; leaky in place; exp on Act
            e = work.tile([128, 2 * KGMAX, HEADS], F16, tag="e1")
            nc.vector.tensor_tensor(
                e[:, 0:K2, :].rearrange("p (g k) h -> p g k h", g=2),
                g[:, 0:K2, 256:264].rearrange("p (g k) h -> p g k h", g=2),
                sdown[:, w0:w0 + 2, :].unsqueeze(2).broadcast_to(
                    [128, 2, Kg, HEADS]),
                OP.add)
            stt(e[:, 0:K2, :], e[:, 0:K2, :], NEG, e[:, 0:K2, :],
                OP.mult, OP.max)
            ex = work.tile([128, 2 * KGMAX, HEADS], F16, tag="ex1")
            nc.scalar.activation(ex[:, 0:K2, :], e[:, 0:K2, :], AF.Exp)
            # msg = h * ex in place over the gathered rows; h is stored
            # channel-major/head-minor so the broadcast stays off the packed
            # innermost axis (keeps the 2x DVE mode)
            nc.vector.tensor_tensor(
                g[:, 0:K2, 0:256].rearrange("p k (c h) -> p k c h", h=HEADS),
                g[:, 0:K2, 0:256].rearrange("p k (c h) -> p k c h", h=HEADS),
                ex[:, 0:K2, :].unsqueeze(2).broadcast_to(
                    [128, K2, HID, HEADS]),
                OP.mult)
            # segment sum: in-place binary fold over the edge axis
            gv = g[:, 0:K2, 0:256].rearrange("p (g k) c -> p g k c", g=2)
            s = Kg
            while s > 1:
                h = s - s // 2
                nc.vector.tensor_tensor(gv[:, :, 0:s // 2, :],
                                        gv[:, :, 0:s // 2, :],
                                        gv[:, :, h:s, :], OP.add)
                s = h
            den = work.tile([128, 2, HEADS], F32, tag="den")
            nc.vector.tensor_reduce(
                den[:], ex[:, 0:K2, :].rearrange("p (g k) h -> p g h k", g=2),
                mybir.AxisListType.X, OP.add)
            denc = work.tile([128, 2, HEADS], F32, tag="denc")
            nc.vector.tensor_scalar(denc[:], den[:], 1e-4, None, OP.max)
            rden = work.tile([128, 2, HEADS], F16, tag="rden")
            with nc.allow_low_precision(reason="1/den rounded to f16; dens "
                                        "are clamped >= 1e-4 so f16 is safe"):
                nc.vector.reciprocal(rden[:], denc[:])
            o1 = work.tile([128, 2, 256], F16, tag="o1")
            nc.vector.tensor_tensor(
                o1[:].rearrange("p g (c h) -> p g c h", h=HEADS),
                gv[:, :, 0, :].rearrange("p g (c h) -> p g c h", h=HEADS),
                rden[:].unsqueeze(2).broadcast_to([128, 2, HID, HEADS]),
                OP.mult)
            h2a = work.tile([128, 2, 256], F16, tag="h2a")
            nc.vector.tensor_tensor(
                h2a[:], o1[:],
                b1t_t[:].unsqueeze(1).broadcast_to([128, 2, 256]), OP.add)
            # elu(x) = exp(min(x,0)) - 1 + x - min(x,0)
            tmin = work.tile([128, 2, 256], F16, tag="tmin")
            nc.vector.tensor_scalar(tmin[:], h2a[:], 0.0, None, OP.min)
            eexp = work.tile([128, 2, 256], F16, tag="eexp")
            nc.scalar.activation(eexp[:], tmin[:], AF.Exp)
            t1 = work.tile([128, 2, 256], F16, tag="t1")
            nc.vector.tensor_tensor(t1[:], eexp[:], h2a[:], OP.add)
            h2e = work.tile([128, 2, 256], F32, tag="h2e")
            stt(h2e[:], t1[:], -1.0, tmin[:], OP.add, OP.subtract)
            # z = h2e @ W2 (+ attention vectors) via transpose + 2 matmuls
            zst2 = work.tile([128, 2, ROW2], F16, tag="zst")
            for i in range(2):
                zps = pp.tile([128, ROW2], F32, tag="zps")
                for half in range(2):
                    trp = pp.tile([128, 128], F32, tag="trp")
                    nc.tensor.transpose(
                        trp[:], h2e[:, i, half * 128:(half + 1) * 128],
                        ident_t[:])
                    h2T = sub.tile([128, 128], F32R, tag="h2T")
                    nc.scalar.activation(h2T[:], trp[:], AF.Copy)
                    nc.tensor.matmul(zps[:], h2T[:],
                                     w2a_t[:, half * ROW2:(half + 1) * ROW2],
                                     start=(half == 0), stop=(half == 1))
                nc.scalar.activation(zst2[:, i, :], zps[:], AF.Copy)
            nc.vector.tensor_copy(sd2own[:, w0:w0 + 2], zst2[:, :, 33])
            nc.sync.dma_start(
                zz_own[w0 * 128:(w0 + GS) * 128, :]
                .rearrange("(i p) c -> p i c", i=GS), zst2[:, 0:GS, :])

        if phases[1]:
            nc.sync.dma_start(zz_own[NPC:GPC, 32:33], padc[0:GPC - NPC, 0:1])

        # ---- all-gather z across cores ----
        if phases[2]:
            if timing:
                # TimelineSim can't model collectives; stand in the same
                # bytes (each core receives NCORES slices) with plain DMAs.
                for c in range(NCORES):
                    nc.sync.dma_start(zz_all[c * GPC:(c + 1) * GPC, :], zz_own)
            else:
                nc.gpsimd.collective_compute(
                    "AllGather", OP.bypass,
                    replica_groups=[list(range(NCORES))],
                    ins=[zz_own], outs=[zz_all])

        # ---- layer 2 (window pairs) ----
        for gi in range(NGRP if phases[2] else 0):
            Kg = K_g[gi]
            off = int(goff[gi])
            w0 = 2 * gi
            GS = 1 if gi == NGRP - 1 else 2
            K2 = 2 * Kg
            zg = gath.tile([128, 2 * KGMAX, ROW2], F16, tag="g2")
            for col in range(K2):
                _gather(zg[:, col, :], zz_all, sidx_t[:, off + col:off + col + 1])
            e2 = work.tile([128, 2 * KGMAX], F32, tag="e2")
            stt(e2[:, 0:K2].rearrange("p (g k) -> p g k", g=2),
                zg[:, 0:K2, 32].rearrange("p (g k) -> p g k", g=2),
                0.0,
                sd2own[:, w0:w0 + 2].unsqueeze(2).broadcast_to([128, 2, Kg]),
                OP.add, OP.add)
            stt(e2[:, 0:K2], e2[:, 0:K2], NEG, e2[:, 0:K2], OP.mult, OP.max)
            ex2 = work.tile([128, 2 * KGMAX], F16, tag="ex2")
            nc.scalar.activation(ex2[:, 0:K2], e2[:, 0:K2], AF.Exp)
            stt(zg[:, 0:K2, 0:32], zg[:, 0:K2, 0:32], 0.0,
                ex2[:, 0:K2].unsqueeze(2).broadcast_to([128, K2, 32]),
                OP.add, OP.mult)
            nc.vector.tensor_reduce(
                num2a[:, w0:w0 + 2, :],
                zg[:, 0:K2, 0:32].rearrange("p (g k) c -> p g c k", g=2),
                mybir.AxisListType.X, OP.add)
            nc.vector.tensor_reduce(
                den2a[:, w0:w0 + 2],
                ex2[:, 0:K2].rearrange("p (g k) -> p g k", g=2),
                mybir.AxisListType.X, OP.add)

        if phases[2]:
            # batched finalize: normalize + bias + log_softmax for all
            # windows at once (keeps Ln off the per-pair Act hot path)
            NWP = NW + 1
            nc.vector.tensor_scalar(den2a[:], den2a[:], 1e-30, None, OP.max)
            rd2 = work.tile([128, NWP], F32, tag="rd2")
            nc.vector.reciprocal(rd2[:], den2a[:])
            stt(num2a[:], num2a[:], 0.0,
                rd2[:].unsqueeze(2).broadcast_to([128, NWP, 32]),
                OP.add, OP.mult)
            stt(num2a[:], num2a[:], 0.0,
                b2t_t[:].unsqueeze(1).broadcast_to([128, NWP, 32]),
                OP.add, OP.add)
            mx = work.tile([128, NWP], F32, tag="mx")
            nc.vector.tensor_reduce(mx[:], num2a[:], mybir.AxisListType.X,
                                    OP.max)
            stt(num2a[:], num2a[:], 0.0,
                mx[:].unsqueeze(2).broadcast_to([128, NWP, 32]),
                OP.add, OP.subtract)
            ew = work.tile([128, NWP, 32], F32, tag="ew")
            nc.scalar.activation(ew[:], num2a[:], AF.Exp)
            ssum = work.tile([128, NWP], F32, tag="ssum")
            nc.vector.tensor_reduce(ssum[:], ew[:], mybir.AxisListType.X,
                                    OP.add)
            lns = work.tile([128, NWP], F32, tag="lns")
            nc.scalar.activation(lns[:], ssum[:], AF.Ln)
            stt(num2a[:], num2a[:], 0.0,
                lns[:].unsqueeze(2).broadcast_to([128, NWP, 32]),
                OP.add, OP.subtract)
            nc.sync.dma_start(
                out_d[:].rearrange("(w p) c -> p w c", p=128),
                num2a[:, 0:NW, :])

    nc.compile()
    return nc


_CACHE = {}


def _get_program(K_g):
    key = ("nc", K_g)
    if key not in _CACHE:
        _CACHE[key] = _build_program(K_g)
    return _CACHE[key]


def _build_timing_program():
    K_g = _CACHE.get("K_g")
    assert K_g is not None, "call kernel() before _build_timing_program()"
    return _build_program(K_g, timing=True)


def _host_arrays(inputs):
    x = np.ascontiguousarray(np.asarray(inputs["x"], dtype=np.float32))
    edge_index = np.asarray(inputs["edge_index"])
    W1 = np.asarray(inputs["W1"], dtype=np.float32)
    as1 = np.asarray(inputs["att_src1"], dtype=np.float32)
    ad1 = np.asarray(inputs["att_dst1"], dtype=np.float32)
    b1 = np.asarray(inputs["b1"], dtype=np.float32)
    W2 = np.asarray(inputs["W2"], dtype=np.float32)
    as2 = np.asarray(inputs["att_src2"], dtype=np.float32)
    ad2 = np.asarray(inputs["att_dst2"], dtype=np.float32)
    b2 = np.asarray(inputs["b2"], dtype=np.float32)

    sidx, ownidx, gid, K_g = _preprocess(edge_index)

    xTw = np.zeros((IN_C, TOT), np.float16)
    xTw[:, gid] = x.T.astype(np.float16)
    # hidden features are stored channel-major/head-minor on device
    W1cm = (W1.reshape(IN_C, HEADS, HID).transpose(0, 2, 1)
            .reshape(IN_C, HEADS * HID))
    A_src = (W1.reshape(IN_C, HEADS, HID) * as1[None]).sum(-1)
    A_dst = (W1.reshape(IN_C, HEADS, HID) * ad1[None]).sum(-1)
    w1a = np.concatenate([W1cm, A_src, A_dst], axis=1).astype(np.float16)
    a2s = W2 @ as2[0]
    a2d = W2 @ ad2[0]
    W2A2 = np.concatenate([W2, a2s[:, None], a2d[:, None]], axis=1)  # [256,34]
    W2A2 = (W2A2.reshape(HEADS, HID, ROW2).transpose(1, 0, 2)
            .reshape(HEADS * HID, ROW2))                # c-major rows
    w2a = np.concatenate([W2A2[0:128], W2A2[128:256]], axis=1).astype(np.float32)
    ident = np.eye(128, dtype=np.float32)
    b1cm = b1.reshape(HEADS, HID).T.reshape(-1)
    b1t = np.tile(b1cm[None, :], (128, 1)).astype(np.float16)
    b2t = np.tile(b2[None, :], (128, 1)).astype(np.float32)

    in_maps = []
    for c in range(NCORES):
        in_maps.append(dict(
            xTw=xTw, w1a=w1a, w2a=w2a, ident=ident, b1t=b1t, b2t=b2t,
            sidx=sidx[c], oidx=ownidx[c],
        ))
    return in_maps, gid, K_g


def kernel(**inputs):
    in_maps, gid, K_g = _host_arrays(inputs)
    _CACHE["K_g"] = K_g
    nc = _get_program(K_g)
    res = run_bass_kernel_spmd(nc, in_maps, core_ids=list(range(NCORES)))
    out_full = np.concatenate(
        [np.asarray(res.results[c]["out2"], dtype=np.float32)
         for c in range(NCORES)], axis=0)
    return out_full[gid]


# revision 17
# speedup vs baseline: 2.3512x; 1.0244x over previous
"""GAT (2-layer, 8-head) forward on 8 Trainium2 NeuronCores via Bass/Tile.

Destination-major strategy: nodes are partitioned across 8 cores (contiguous
ranges of 6250). Within a core, nodes are sorted by in-degree and packed into
49 windows of 128 nodes; a window's nodes sit on SBUF partitions and their
incoming edges lie along the free axis, padded to the window max degree
(degree-sorted packing keeps padding ~3%). Windows are processed in PAIRS
with a shared padded degree Kg so every compute op covers [128, 2, Kg, ...]:
  - indirect row gathers fetch [h | s_src | s_dst] source rows (f16, 544B)
    one 128-index column at a time (the HW SWDGE supports exactly one index
    per partition per indirect DMA; its ~1us/instr prep is the bottleneck).
  - dst attention scores are per-partition values (broadcast for free) --
    no per-edge dst-score gathers and no one-hot scatter matmuls at all.
  - messages are formed in place over the gathered rows (h is stored
    channel-major so the ex broadcast stays off the packed innermost axis,
    keeping the 2x DVE mode) and segment-summed by an in-place binary fold
    over the edge axis.
Layer-1 node features h1 = x@W1 (+ fused score vectors) are computed
replicated on every core into a local f16 table; layer 2 aggregates 34-float
z rows that are AllGather'ed across cores. log_softmax runs as one batched
epilogue over all windows (keeps Ln off the per-pair Act hot path).
"""
import sys

sys.path.insert(0, "/opt/trn_rl_repo")

import numpy as np
from contextlib import ExitStack

import concourse.bass as bass
import concourse.tile as tile
from concourse import bacc, mybir
from concourse.bass_utils import run_bass_kernel_spmd

F16 = mybir.dt.float16
F32 = mybir.dt.float32
F32R = mybir.dt.float32r
I32 = mybir.dt.int32
AF = mybir.ActivationFunctionType
OP = mybir.AluOpType

# problem constants (hardcoded per contract)
N = 50000
E = 800000
IN_C = 128
HID = 32
HEADS = 8
OUT_C = 32
NEG = 0.2

NCORES = 8
NPC = N // NCORES           # 6250 nodes per core
NW = 49                     # windows per core
GPC = NW * 128              # 6272 slots per core (incl 22 pads in window 48)
TOT = NCORES * GPC          # 50176
NT0 = TOT // 128            # 392 phase-0 tiles
NGRP = (NW + 1) // 2        # 25 window pairs (last one is a singleton)
PAD_FILL = -200.0           # pad source score -> exp(leaky(.)) flushes to 0
PADP = NPC - 48 * 128       # 106: first pad partition of window 48
ROW1 = HEADS * HID + 2 * HEADS   # 272: h(256) | s_src(8) | s_dst(8)
ROW2 = OUT_C + 2                 # 34: z(32) | s_src2 | s_dst2


# ----------------------------------------------------------------------------
# host preprocessing
# ----------------------------------------------------------------------------

def _preprocess(edge_index):
    src = np.concatenate([np.asarray(edge_index[0], np.int64),
                          np.arange(N, dtype=np.int64)])
    dst = np.concatenate([np.asarray(edge_index[1], np.int64),
                          np.arange(N, dtype=np.int64)])
    deg = np.bincount(dst, minlength=N)          # >= 1 (self-loops)

    gid = np.empty(N, np.int64)
    K_cw = np.zeros((NCORES, NW), np.int64)
    for c in range(NCORES):
        d = deg[c * NPC:(c + 1) * NPC]
        order = np.argsort(-d, kind="stable")
        ranks = np.empty(NPC, np.int64)
        ranks[order] = np.arange(NPC)
        gid[c * NPC:(c + 1) * NPC] = c * GPC + ranks
        ds = d[order]                            # descending
        K_cw[c] = ds[np.minimum(np.arange(NW) * 128, NPC - 1)]
    K_w = np.maximum(K_cw.max(axis=0), 1)
    # pair windows (2g, 2g+1) under a shared padded degree Kg
    K_g = np.array([max(K_w[2 * g], K_w[2 * g + 1]) if 2 * g + 1 < NW
                    else K_w[2 * g] for g in range(NGRP)], np.int64)
    goff = np.zeros(NGRP + 1, np.int64)
    goff[1:] = np.cumsum(2 * K_g)
    NCOL = int(goff[-1])

    # place every edge at (core, dst_slot, goff[g] + j*Kg + rank_in_node)
    Etot = dst.shape[0]
    eorder = np.argsort(dst, kind="stable")
    dst_s = dst[eorder]
    src_s = src[eorder]
    row_start = np.zeros(N, np.int64)
    row_start[1:] = np.cumsum(deg)[:-1]
    pos = np.arange(Etot) - row_start[dst_s]
    c_e = dst_s // NPC
    loc = gid[dst_s] - c_e * GPC
    w_e = loc // 128
    s_e = loc % 128
    g_e = w_e // 2
    col = goff[g_e] + (w_e % 2) * K_g[g_e] + pos
    assert (pos < K_w[w_e]).all()

    sidx = np.empty((NCORES, 128, NCOL), np.int32)
    for c in range(NCORES):
        sidx[c] = c * GPC + GPC - 1              # dummy pad row (h=0, s=-200)
    sidx[c_e, s_e, col] = gid[src_s].astype(np.int32)

    ownidx = np.empty((NCORES, 128, NW + 1), np.int32)
    base = (np.arange(NW)[None, :] * 128 + np.arange(128)[:, None])
    for c in range(NCORES):
        ownidx[c, :, 0:NW] = c * GPC + base
        ownidx[c, :, NW] = c * GPC + GPC - 1     # phantom sub-window
    return (sidx, ownidx, gid, tuple(int(k) for k in K_g),
            tuple(int(k) for k in K_w))


# ----------------------------------------------------------------------------
# bass program
# ----------------------------------------------------------------------------

def _build_program(K_g, K_w, timing=False, phases=(1, 1, 1)):
    KGMAX = max(K_g)
    goff = np.concatenate([[0], np.cumsum([2 * k for k in K_g])]).astype(int)
    NCOL = int(goff[-1])

    nc = bacc.Bacc("TRN2", target_bir_lowering=False, debug=False,
                   num_devices=NCORES, num_swdge_queues=4)

    def _gather(out_ap, table, idx_ap, element_offset=0):
        return nc.gpsimd.indirect_dma_start(
            out=out_ap, out_offset=None, in_=table,
            in_offset=bass.IndirectOffsetOnAxis(ap=idx_ap, axis=0),
            element_offset=element_offset)

    def stt(out, in0, scalar, in1, op0, op1):
        nc.vector.scalar_tensor_tensor(out, in0, scalar, in1, op0, op1)

    xTw_d = nc.dram_tensor("xTw", [IN_C, TOT], F16, kind="ExternalInput").ap()
    w1a_d = nc.dram_tensor("w1a", [IN_C, ROW1], F16, kind="ExternalInput").ap()
    w2a_d = nc.dram_tensor("w2a", [128, 2 * ROW2], F32R, kind="ExternalInput").ap()
    ident_d = nc.dram_tensor("ident", [128, 128], F32, kind="ExternalInput").ap()
    b1t_d = nc.dram_tensor("b1t", [128, 256], F16, kind="ExternalInput").ap()
    b2t_d = nc.dram_tensor("b2t", [128, 32], F32, kind="ExternalInput").ap()
    sidx_d = nc.dram_tensor("sidx", [128, NCOL], I32, kind="ExternalInput").ap()
    oidx_d = nc.dram_tensor("oidx", [128, NW + 1], I32, kind="ExternalInput").ap()

    out_d = nc.dram_tensor("out2", [GPC, OUT_C], F32, kind="ExternalOutput").ap()

    h1tab = nc.dram_tensor("h1tab", [TOT, ROW1], F16, kind="Internal").ap()
    zz_own = nc.dram_tensor("zz_own", [GPC, ROW2], F16, kind="Internal").ap()
    zz_all = nc.dram_tensor("zz_all", [TOT, ROW2], F16, kind="Internal",
                            addr_space="Shared").ap()

    with tile.TileContext(nc) as tc, ExitStack() as ctx:
        cons = ctx.enter_context(tc.tile_pool(name="cons", bufs=1))
        stat = ctx.enter_context(tc.tile_pool(name="stat", bufs=3))
        gath = ctx.enter_context(tc.tile_pool(name="gath", bufs=2))
        work = ctx.enter_context(tc.tile_pool(name="work", bufs=3))
        sub = ctx.enter_context(tc.tile_pool(name="sub", bufs=3))
        pp = ctx.enter_context(tc.tile_pool(name="pp", bufs=2, space="PSUM"))

        # ---- constants resident in SBUF ----
        w1a_t = cons.tile([IN_C, ROW1], F16)
        nc.sync.dma_start(w1a_t[:], w1a_d)
        w2a_t = cons.tile([128, 2 * ROW2], F32R)
        nc.sync.dma_start(w2a_t[:], w2a_d)
        ident_t = cons.tile([128, 128], F32)
        nc.sync.dma_start(ident_t[:], ident_d)
        b1t_t = cons.tile([128, 256], F16)
        nc.sync.dma_start(b1t_t[:], b1t_d)
        b2t_t = cons.tile([128, 32], F32)
        nc.sync.dma_start(b2t_t[:], b2t_d)
        sidx_t = cons.tile([128, NCOL], I32)
        nc.sync.dma_start(sidx_t[:], sidx_d)
        oidx_t = cons.tile([128, NW + 1], I32)
        nc.sync.dma_start(oidx_t[:], oidx_d)
        sdown = cons.tile([128, NW + 1, HEADS], F16)   # own s_dst, layer 1
        sd2own = cons.tile([128, NW + 1], F32)         # own s_dst, layer 2
        num2a = cons.tile([128, NW + 1, 32], F32)      # layer-2 numerators
        den2a = cons.tile([128, NW + 1], F32)          # layer-2 denominators
        padc = cons.tile([128, 16], F16)
        nc.vector.memset(padc[:], PAD_FILL)

        # ---- phase 0: h1 table [TOT, 272] f16 = [h | s_src | s_dst] ----
        # 8 node-tiles per DMA load and per DMA store; PSUM in pairs; the
        # PSUM->SBUF f16 copies alternate between Act and DVE.
        for t8 in range(NT0 // 8 if phases[0] else 0):
            xt8 = stat.tile([IN_C, 8, 128], F16, tag="xt8")
            nc.sync.dma_start(xt8[:], xTw_d[:, t8 * 1024:(t8 + 1) * 1024])
            stg8 = work.tile([128, 8, ROW1], F16, tag="stg0")
            for q in range(4):
                ph2 = pp.tile([128, 2, 512], F32, tag="ph0")
                for i in range(2):
                    nc.tensor.matmul(ph2[:, i, 0:ROW1], xt8[:, 2 * q + i, :],
                                     w1a_t[:], start=True, stop=True)
                if q % 2 == 0:
                    nc.scalar.activation(stg8[:, 2 * q:2 * q + 2, :],
                                         ph2[:, :, 0:ROW1], AF.Copy)
                else:
                    nc.vector.tensor_copy(stg8[:, 2 * q:2 * q + 2, :],
                                          ph2[:, :, 0:ROW1])
            nc.sync.dma_start(
                h1tab[t8 * 1024:(t8 + 1) * 1024, :]
                .rearrange("(i p) c -> p i c", i=8), stg8[:])

        if phases[0]:
            for c in range(NCORES):
                nc.sync.dma_start(
                    h1tab[c * GPC + NPC:(c + 1) * GPC, 256:ROW1],
                    padc[0:GPC - NPC, :])

        if phases[1]:
            # own per-node dst scores -> SBUF (one tiny batched gather)
            for w in range(NW + 1):
                _gather(sdown[:, w, :], h1tab, oidx_t[:, w:w + 1],
                        element_offset=264)

        # ---- layer 1 (window pairs) ----
        for gi in range(NGRP if phases[1] else 0):
            Kg = K_g[gi]
            off = int(goff[gi])
            w0 = 2 * gi
            GS = 1 if gi == NGRP - 1 else 2
            K2 = 2 * Kg
            g = gath.tile([128, 2 * KGMAX, ROW1], F16, tag="g1")
            for j in range(2):
                real = K_w[w0 + j] if w0 + j < NW else 0
                for k in range(real):
                    col = j * Kg + k
                    _gather(g[:, col, :], h1tab,
                            sidx_t[:, off + col:off + col + 1])
                if real < Kg:
                    # ungathered tail columns: fill whole rows so ex
                    # underflows to 0 and msg = 0 * finite = 0 (initial
                    # SBUF bytes could decode as NaN/Inf otherwise)
                    nc.vector.memset(
                        g[:, j * Kg + real:(j + 1) * Kg, :], PAD_FILL)
            # e = s_src[src] + s_dst[dst] ; leaky in place; exp on Act
            e = work.tile([128, 2 * KGMAX, HEADS], F16, tag="e1")
            nc.vector.tensor_tensor(
                e[:, 0:K2, :].rearrange("p (g k) h -> p g k h", g=2),
                g[:, 0:K2, 256:264].rearrange("p (g k) h -> p g k h", g=2),
                sdown[:, w0:w0 + 2, :].unsqueeze(2).broadcast_to(
                    [128, 2, Kg, HEADS]),
                OP.add)
            stt(e[:, 0:K2, :], e[:, 0:K2, :], NEG, e[:, 0:K2, :],
                OP.mult, OP.max)
            ex = work.tile([128, 2 * KGMAX, HEADS], F16, tag="ex1")
            nc.scalar.activation(ex[:, 0:K2, :], e[:, 0:K2, :], AF.Exp)
            # msg = h * ex in place over the gathered rows; h is stored
            # channel-major/head-minor so the broadcast stays off the packed
            # innermost axis (keeps the 2x DVE mode)
            nc.vector.tensor_tensor(
                g[:, 0:K2, 0:256].rearrange("p k (c h) -> p k c h", h=HEADS),
                g[:, 0:K2, 0:256].rearrange("p k (c h) -> p k c h", h=HEADS),
                ex[:, 0:K2, :].unsqueeze(2).broadcast_to(
                    [128, K2, HID, HEADS]),
                OP.mult)
            # segment sum: in-place binary fold over the edge axis
            gv = g[:, 0:K2, 0:256].rearrange("p (g k) c -> p g k c", g=2)
            s = Kg
            while s > 1:
                h = s - s // 2
                nc.vector.tensor_tensor(gv[:, :, 0:s // 2, :],
                                        gv[:, :, 0:s // 2, :],
                                        gv[:, :, h:s, :], OP.add)
                s = h
            den = work.tile([128, 2, HEADS], F32, tag="den")
            nc.vector.tensor_reduce(
                den[:], ex[:, 0:K2, :].rearrange("p (g k) h -> p g h k", g=2),
                mybir.AxisListType.X, OP.add)
            denc = work.tile([128, 2, HEADS], F32, tag="denc")
            nc.vector.tensor_scalar(denc[:], den[:], 1e-4, None, OP.max)
            rden = work.tile([128, 2, HEADS], F16, tag="rden")
            with nc.allow_low_precision(reason="1/den rounded to f16; dens "
                                        "are clamped >= 1e-4 so f16 is safe"):
                nc.vector.reciprocal(rden[:], denc[:])
            o1 = work.tile([128, 2, 256], F16, tag="o1")
            nc.vector.tensor_tensor(
                o1[:].rearrange("p g (c h) -> p g c h", h=HEADS),
                gv[:, :, 0, :].rearrange("p g (c h) -> p g c h", h=HEADS),
                rden[:].unsqueeze(2).broadcast_to([128, 2, HID, HEADS]),
                OP.mult)
            h2a = work.tile([128, 2, 256], F16, tag="h2a")
            nc.vector.tensor_tensor(
                h2a[:], o1[:],
                b1t_t[:].unsqueeze(1).broadcast_to([128, 2, 256]), OP.add)
            # elu(x) = exp(min(x,0)) - 1 + x - min(x,0)
            tmin = work.tile([128, 2, 256], F16, tag="tmin")
            nc.vector.tensor_scalar(tmin[:], h2a[:], 0.0, None, OP.min)
            eexp = work.tile([128, 2, 256], F16, tag="eexp")
            nc.scalar.activation(eexp[:], tmin[:], AF.Exp)
            t1 = work.tile([128, 2, 256], F16, tag="t1")
            nc.vector.tensor_tensor(t1[:], eexp[:], h2a[:], OP.add)
            h2e = work.tile([128, 2, 256], F32, tag="h2e")
            stt(h2e[:], t1[:], -1.0, tmin[:], OP.add, OP.subtract)
            # z = h2e @ W2 (+ attention vectors) via transpose + 2 matmuls
            zst2 = work.tile([128, 2, ROW2], F16, tag="zst")
            for i in range(2):
                zps = pp.tile([128, ROW2], F32, tag="zps")
                for half in range(2):
                    trp = pp.tile([128, 128], F32, tag="trp")
                    nc.tensor.transpose(
                        trp[:], h2e[:, i, half * 128:(half + 1) * 128],
                        ident_t[:])
                    h2T = sub.tile([128, 128], F32R, tag="h2T")
                    nc.scalar.activation(h2T[:], trp[:], AF.Copy)
                    nc.tensor.matmul(zps[:], h2T[:],
                                     w2a_t[:, half * ROW2:(half + 1) * ROW2],
                                     start=(half == 0), stop=(half == 1))
                nc.scalar.activation(zst2[:, i, :], zps[:], AF.Copy)
            nc.vector.tensor_copy(sd2own[:, w0:w0 + 2], zst2[:, :, 33])
            nc.sync.dma_start(
                zz_own[w0 * 128:(w0 + GS) * 128, :]
                .rearrange("(i p) c -> p i c", i=GS), zst2[:, 0:GS, :])

        if phases[1]:
            nc.sync.dma_start(zz_own[NPC:GPC, 32:33], padc[0:GPC - NPC, 0:1])

        # ---- all-gather z across cores ----
        if phases[2]:
            if timing:
                # TimelineSim can't model collectives; stand in the same
                # bytes (each core receives NCORES slices) with plain DMAs.
                for c in range(NCORES):
                    nc.sync.dma_start(zz_all[c * GPC:(c + 1) * GPC, :], zz_own)
            else:
                nc.gpsimd.collective_compute(
                    "AllGather", OP.bypass,
                    replica_groups=[list(range(NCORES))],
                    ins=[zz_own], outs=[zz_all])

        # ---- layer 2 (window pairs) ----
        for gi in range(NGRP if phases[2] else 0):
            Kg = K_g[gi]
            off = int(goff[gi])
            w0 = 2 * gi
            GS = 1 if gi == NGRP - 1 else 2
            K2 = 2 * Kg
            zg = gath.tile([128, 2 * KGMAX, ROW2], F16, tag="g2")
            for j in range(2):
                real = K_w[w0 + j] if w0 + j < NW else 0
                for k in range(real):
                    col = j * Kg + k
                    _gather(zg[:, col, :], zz_all,
                            sidx_t[:, off + col:off + col + 1])
                if real < Kg:
                    nc.vector.memset(
                        zg[:, j * Kg + real:(j + 1) * Kg, :], PAD_FILL)
            e2 = work.tile([128, 2 * KGMAX], F32, tag="e2")
            stt(e2[:, 0:K2].rearrange("p (g k) -> p g k", g=2),
                zg[:, 0:K2, 32].rearrange("p (g k) -> p g k", g=2),
                0.0,
                sd2own[:, w0:w0 + 2].unsqueeze(2).broadcast_to([128, 2, Kg]),
                OP.add, OP.add)
            stt(e2[:, 0:K2], e2[:, 0:K2], NEG, e2[:, 0:K2], OP.mult, OP.max)
            ex2 = work.tile([128, 2 * KGMAX], F16, tag="ex2")
            nc.scalar.activation(ex2[:, 0:K2], e2[:, 0:K2], AF.Exp)
            stt(zg[:, 0:K2, 0:32], zg[:, 0:K2, 0:32], 0.0,
                ex2[:, 0:K2].unsqueeze(2).broadcast_to([128, K2, 32]),
                OP.add, OP.mult)
            nc.vector.tensor_reduce(
                num2a[:, w0:w0 + 2, :],
                zg[:, 0:K2, 0:32].rearrange("p (g k) c -> p g c k", g=2),
                mybir.AxisListType.X, OP.add)
            nc.vector.tensor_reduce(
                den2a[:, w0:w0 + 2],
                ex2[:, 0:K2].rearrange("p (g k) -> p g k", g=2),
                mybir.AxisListType.X, OP.add)

        if phases[2]:
            # batched finalize: normalize + bias + log_softmax for all
            # windows at once (keeps Ln off the per-pair Act hot path)
            NWP = NW + 1
            nc.vector.tensor_scalar(den2a[:], den2a[:], 1e-30, None, OP.max)
            rd2 = work.tile([128, NWP], F32, tag="rd2")
            nc.vector.reciprocal(rd2[:], den2a[:])
            stt(num2a[:], num2a[:], 0.0,
                rd2[:].unsqueeze(2).broadcast_to([128, NWP, 32]),
                OP.add, OP.mult)
            stt(num2a[:], num2a[:], 0.0,
                b2t_t[:].unsqueeze(1).broadcast_to([128, NWP, 32]),
                OP.add, OP.add)
            mx = work.tile([128, NWP], F32, tag="mx")
            nc.vector.tensor_reduce(mx[:], num2a[:], mybir.AxisListType.X,
                                    OP.max)
            stt(num2a[:], num2a[:], 0.0,
                mx[:].unsqueeze(2).broadcast_to([128, NWP, 32]),
                OP.add, OP.subtract)
            ew = work.tile([128, NWP, 32], F32, tag="ew")
            nc.scalar.activation(ew[:], num2a[:], AF.Exp)
            ssum = work.tile([128, NWP], F32, tag="ssum")
            nc.vector.tensor_reduce(ssum[:], ew[:], mybir.AxisListType.X,
                                    OP.add)
            lns = work.tile([128, NWP], F32, tag="lns")
            nc.scalar.activation(lns[:], ssum[:], AF.Ln)
            stt(num2a[:], num2a[:], 0.0,
                lns[:].unsqueeze(2).broadcast_to([128, NWP, 32]),
                OP.add, OP.subtract)
            nc.sync.dma_start(
                out_d[:].rearrange("(w p) c -> p w c", p=128),
                num2a[:, 0:NW, :])

    nc.compile()
    return nc


_CACHE = {}


def _get_program(K_g, K_w):
    key = ("nc", K_g, K_w)
    if key not in _CACHE:
        _CACHE[key] = _build_program(K_g, K_w)
    return _CACHE[key]


def _build_timing_program():
    K_g, K_w = _CACHE.get("K_gw", (None, None))
    assert K_g is not None, "call kernel() before _build_timing_program()"
    return _build_program(K_g, K_w, timing=True)


def _host_arrays(inputs):
    x = np.ascontiguousarray(np.asarray(inputs["x"], dtype=np.float32))
    edge_index = np.asarray(inputs["edge_index"])
    W1 = np.asarray(inputs["W1"], dtype=np.float32)
    as1 = np.asarray(inputs["att_src1"], dtype=np.float32)
    ad1 = np.asarray(inputs["att_dst1"], dtype=np.float32)
    b1 = np.asarray(inputs["b1"], dtype=np.float32)
    W2 = np.asarray(inputs["W2"], dtype=np.float32)
    as2 = np.asarray(inputs["att_src2"], dtype=np.float32)
    ad2 = np.asarray(inputs["att_dst2"], dtype=np.float32)
    b2 = np.asarray(inputs["b2"], dtype=np.float32)

    sidx, ownidx, gid, K_g, K_w = _preprocess(edge_index)

    xTw = np.zeros((IN_C, TOT), np.float16)
    xTw[:, gid] = x.T.astype(np.float16)
    # hidden features are stored channel-major/head-minor on device
    W1cm = (W1.reshape(IN_C, HEADS, HID).transpose(0, 2, 1)
            .reshape(IN_C, HEADS * HID))
    A_src = (W1.reshape(IN_C, HEADS, HID) * as1[None]).sum(-1)
    A_dst = (W1.reshape(IN_C, HEADS, HID) * ad1[None]).sum(-1)
    w1a = np.concatenate([W1cm, A_src, A_dst], axis=1).astype(np.float16)
    a2s = W2 @ as2[0]
    a2d = W2 @ ad2[0]
    W2A2 = np.concatenate([W2, a2s[:, None], a2d[:, None]], axis=1)  # [256,34]
    W2A2 = (W2A2.reshape(HEADS, HID, ROW2).transpose(1, 0, 2)
            .reshape(HEADS * HID, ROW2))                # c-major rows
    w2a = np.concatenate([W2A2[0:128], W2A2[128:256]], axis=1).astype(np.float32)
    ident = np.eye(128, dtype=np.float32)
    b1cm = b1.reshape(HEADS, HID).T.reshape(-1)
    b1t = np.tile(b1cm[None, :], (128, 1)).astype(np.float16)
    b2t = np.tile(b2[None, :], (128, 1)).astype(np.float32)

    in_maps = []
    for c in range(NCORES):
        in_maps.append(dict(
            xTw=xTw, w1a=w1a, w2a=w2a, ident=ident, b1t=b1t, b2t=b2t,
            sidx=sidx[c], oidx=ownidx[c],
        ))
    return in_maps, gid, K_g, K_w


def kernel(**inputs):
    in_maps, gid, K_g, K_w = _host_arrays(inputs)
    _CACHE["K_gw"] = (K_g, K_w)
    nc = _get_program(K_g, K_w)
    res = run_bass_kernel_spmd(nc, in_maps, core_ids=list(range(NCORES)))
    out_full = np.concatenate(
        [np.asarray(res.results[c]["out2"], dtype=np.float32)
         for c in range(NCORES)], axis=0)
    return out_full[gid]
